# revision 1
# baseline (speedup 1.0000x reference)
"""Trainium2 Bass kernel for nn_Attention_43542378447097.

GroupNorm -> multi-head causal self-attention -> out-proj, then the
reference's broadcast add:

    out(B,S,C) + residual(B,C,1,C)  ->  (B,C,S,C)   [right-aligned numpy
    broadcasting, so batches MIX]:

    result[i, j, k, l] = A[j, k, l] + xn[i, j, l]

where A[j] = attention output (incl bo) of batch j and xn[i] = groupnorm
output of batch i.  Output is (96, 96, 96, 96), ~85M elements -> memory
bound on the output write.

v2 strategy (vs the v1 i-sharded fp32 kernel):
  * Shard over j (A's batch index) instead of i: core c owns
    j in J = [12c, 12c+12).  It needs (a) full attention for its own 12
    batches (local), and (b) xn[i, J, :] for ALL 96 batches i -- but the
    groupnorm groups are 8 groups of 12 s-rows, exactly matching the
    shard, so slice (b) needs only group-c stats from x[:, J, :]
    (442 KB).  NO collective at all, and output writes for j-pair t
    start as soon as local batches 2t, 2t+1 finish -> the big output
    DMA overlaps all remaining compute.
  * fp16 everywhere on the wide paths: PE matmuls run 1 cycle/row in
    fp16 vs 4 for fp32; the output is written as fp16 (half the DMA
    bytes; global rel err ~1e-3, well under the 2e-2 gate) and widened
    to fp32 on the host.
  * Assembly layout [l | i, k, j]: partition dim = l is the only dim
    both addends depend on, so both operands are per-partition with
    mid-dim stride-0 broadcasts only (i for the A term, k for the xn
    term) and a contiguous innermost j -> DVE runs in 2x packed mode.
    out[l, i, k, jpair] = Aarr[l, -, k, j] + Xarr[l, i, -, j].
  * Per-core output (6, 96, 96, 96, 2) fp16 [jp, l, i, k, j2], host
    transposes to (i, j, k, l).

Attention per local batch (fp16 PE, fp32 stats):
  xnT (97 rows: xn^T + ones row; q/k biases are a 97th contraction row),
  q/k per head via lhsT=W-slice, v via lhsT=xnT; scoresT = kT_h.T@qT_h
  -> ACT exp -> causal mask mult -> denominators via ones-matmul ->
  reciprocal -> attnT -> oT = v^T@attnT -> A_jT[l,k] = sum_h Wo_h^T@oT_h
  (note lhsT=wo gives A transposed, exactly what assembly needs).
  1/sqrt(dk) folded into Wq/bq, bv folded into bo_eff = bv@Wo + bo,
  bo_eff + beta folded into the Xarr eviction; groupnorm rsqrt is an
  all-DVE Newton iteration so ACT only ever loads the Exp table.
"""

import sys

sys.path.insert(0, "/opt/trn_rl_repo")

import numpy as np

B_TOTAL = 96
C = 96
S = 96
NH = 8
DK = 96
G = 8
NCORES = 8
JPC = B_TOTAL // NCORES  # 12 j's (= local attention batches) per core
NJP = JPC // 2  # 6 j-pairs
EPS = 1e-5

_PROG = None


ASPL = 48  # assembly i-split: DVE rows [0, ASPL), GpSimd rows [ASPL, 48)


def _build_program(loop_n=1, phases="123", skip_collective=True, aspl=None):
    import contextlib

    import concourse.bass as bass
    import concourse.tile as tile
    from concourse import bacc, mybir

    f32 = mybir.dt.float32
    f16 = mybir.dt.float16
    AF = mybir.ActivationFunctionType
    ALU = mybir.AluOpType
    AX = mybir.AxisListType

    aspl = ASPL if aspl is None else aspl

    nc = bacc.Bacc(
        "TRN2",
        target_bir_lowering=False,
        debug=False,
        enable_asserts=False,
        num_devices=NCORES,
    )

    xall_d = nc.declare_dram_parameter("xall", [S, JPC, C], f16, isOutput=False)
    xg_d = nc.declare_dram_parameter("xg", [B_TOTAL, JPC * C], f16, isOutput=False)
    # all f16 / f32 constants packed into one tensor each: a dozen tiny
    # serialized const DMAs at startup cost ~7 us of dead time otherwise
    cp16_d = nc.declare_dram_parameter("cpack16", [C + 1, 3840], f16, isOutput=False)
    cp32_d = nc.declare_dram_parameter("cpack32", [C, 26], f32, isOutput=False)
    out_d = nc.declare_dram_parameter(
        "out", [NJP, C, B_TOTAL, S, 2], f16, isOutput=True
    )

    with tile.TileContext(nc) as tc:
        with (
            tc.tile_pool(name="const", bufs=1) as cpool,
            tc.tile_pool(name="work", bufs=2) as work,
            tc.tile_pool(name="psum", bufs=6, space="PSUM") as pp,
        ):
            # ---- constants (two packed tiles; views below) ----
            cp16 = cpool.tile([C + 1, 3840], f16, name="cp16")
            cp32 = cpool.tile([C, 26], f32, name="cp32")
            xarr = cpool.tile([C, B_TOTAL, JPC], f16, name="xarr")
            aarr = cpool.tile([C, S, JPC], f16, name="aarr")

            wq_sb = cp16[:, 0:768].rearrange("p (h d) -> p h d", h=NH)
            wk_sb = cp16[:, 768:1536].rearrange("p (h d) -> p h d", h=NH)
            wv_sb = cp16[0:C, 1536:2304].rearrange("p (h d) -> p h d", h=NH)
            wo_sb = cp16[0:DK, 2304:3072].rearrange("p (h d) -> p h d", h=NH)
            gmask_sb = cp16[0:C, 3072:3168]
            ones_sb = cp16[0:S, 3168:3264]
            maskb_sb = cp16[0:S, 3264:3360]
            iden4_sb = cp16[0:C, 3360:3744].rearrange("p (h q) -> p h q", h=4)
            iden_sb = cp16[0:C, 3744:3840]
            gamma_sb = cp32[:, 0:1]
            beta_sb = cp32[:, 1:2]
            gvec_sb = cp32[:, 2:14]
            bb_sb = cp32[:, 14:26]

            nc.sync.dma_start(out=cp16, in_=cp16_d[:])
            nc.sync.dma_start(out=cp32, in_=cp32_d[:])

            inv_na = 1.0 / (C * C // G)  # 1/1152, attention-side groups
            i32 = mybir.dt.int32

            loop_cm = (
                tc.For_i(0, loop_n, 1) if loop_n > 1 else contextlib.nullcontext()
            )
            loop_cm.__enter__()

            def newton_rsqrt(veps, tag, iters=2):
                """rstd = rsqrt(veps), all-DVE (quake seed + Newton steps)
                so ACT only ever needs the Exp table set."""
                shp = list(veps.shape)
                iv = veps.bitcast(i32)
                ineg = work.tile(shp, i32, tag="sti", bufs=8, name="ineg" + tag)
                nc.vector.tensor_scalar_mul(ineg, iv, -1)
                nc.vector.tensor_scalar(ineg, ineg, 1, None, op0=ALU.arith_shift_right)
                nc.vector.tensor_scalar(ineg, ineg, 0x5F3759DF, None, op0=ALU.add)
                y = ineg.bitcast(f32)
                t1 = work.tile(shp, f32, tag="st", bufs=8, name="t1" + tag)
                for _ in range(iters):
                    nc.vector.tensor_mul(t1, y, y)
                    nc.vector.tensor_mul(t1, t1, veps)
                    nc.vector.tensor_scalar(t1, t1, -0.5, 1.5, op0=ALU.mult, op1=ALU.add)
                    nc.vector.tensor_mul(y, y, t1)
                return y

            # ===== prologue 2: groupnorm stats for ALL 12 attention batches
            # (one [C, 12] micro-chain); per-batch xn becomes a single ACT
            # Identity op so the in-loop attention chain never touches DVE
            # except the softmax reciprocal.
            xall_sb = cpool.tile([S, JPC, C], f16, name="xall_sb")
            scale_tb = cpool.tile([C, JPC], f32, name="scale_tb")
            shift_tb = cpool.tile([C, JPC], f32, name="shift_tb")
            if "2" in phases:
                nc.sync.dma_start(out=xall_sb, in_=xall_d[:])
                x2all = work.tile([S, JPC, C], f16, tag="x2all", name="x2all")
                nc.vector.tensor_mul(x2all, xall_sb, xall_sb)
                s12v = work.tile([C, JPC], f32, tag="stv", bufs=8, name="s12v")
                s22v = work.tile([C, JPC], f32, tag="stv", bufs=8, name="s22v")
                for g in range(3):
                    bs = slice(4 * g, 4 * (g + 1))
                    ps1 = pp.tile([C, 4 * C], f32, tag="pss", bufs=2, name="ps_s1")
                    nc.tensor.matmul(
                        ps1,
                        lhsT=gmask_sb,
                        rhs=xall_sb[:, bs, :].rearrange("p b c -> p (b c)"),
                        start=True,
                        stop=True,
                    )
                    nc.vector.tensor_reduce(
                        out=s12v[:, bs].unsqueeze(2),
                        in_=ps1.rearrange("p (b c) -> p b c", b=4),
                        axis=AX.X,
                        op=ALU.add,
                    )
                    ps2 = pp.tile([C, 4 * C], f32, tag="pss", bufs=2, name="ps_s2")
                    nc.tensor.matmul(
                        ps2,
                        lhsT=gmask_sb,
                        rhs=x2all[:, bs, :].rearrange("p b c -> p (b c)"),
                        start=True,
                        stop=True,
                    )
                    nc.vector.tensor_reduce(
                        out=s22v[:, bs].unsqueeze(2),
                        in_=ps2.rearrange("p (b c) -> p b c", b=4),
                        axis=AX.X,
                        op=ALU.add,
                    )
                mu = work.tile([C, JPC], f32, tag="stv", bufs=8, name="mu")
                ex2 = work.tile([C, JPC], f32, tag="stv", bufs=8, name="ex2")
                nc.vector.tensor_scalar_mul(mu, s12v, inv_na)
                nc.vector.tensor_scalar_mul(ex2, s22v, inv_na)
                musq = work.tile([C, JPC], f32, tag="stv", bufs=8, name="musq")
                nc.vector.tensor_mul(musq, mu, mu)
                veps = work.tile([C, JPC], f32, tag="stv", bufs=8, name="veps")
                nc.vector.scalar_tensor_tensor(
                    veps, ex2, EPS, musq, op0=ALU.add, op1=ALU.subtract
                )
                y = newton_rsqrt(veps, "a", iters=1)
                nc.vector.tensor_mul(
                    scale_tb, y, gamma_sb.to_broadcast((C, JPC))
                )
                mus = work.tile([C, JPC], f32, tag="stv", bufs=8, name="mus")
                nc.vector.tensor_mul(mus, mu, scale_tb)
                nc.vector.tensor_sub(
                    shift_tb, beta_sb.to_broadcast((C, JPC)), mus
                )

            # ===== prologue 1: xn slices (group-c rows J, ALL 96 batches) ====
            # xarr[l, i, j] = xn[i, 12c+j, l] * gamma[12c+j] + beta[..] + bo_eff[l]
            # The 12 transpose+evict steps are NOT emitted here; xarr_pair(t)
            # below feeds them into the pair loop just-in-time (asm(t) only
            # reads xarr[:, :, 2t:2t+2]).
            xn_s = cpool.tile([B_TOTAL, JPC, C], f16, name="xn_s")
            if "1" in phases:
                xg_sb = work.tile([B_TOTAL, JPC * C], f16, tag="xg", name="xg_sb")
                nc.sync.dma_start(out=xg_sb, in_=xg_d[:])
                sq = work.tile([B_TOTAL, JPC * C], f16, tag="sq", name="sq")
                nc.vector.tensor_mul(sq, xg_sb, xg_sb)
                s1 = work.tile([C, 1], f32, tag="st", bufs=8, name="s1g")
                s2 = work.tile([C, 1], f32, tag="st", bufs=8, name="s2g")
                nc.vector.tensor_reduce(out=s1, in_=xg_sb, axis=AX.X, op=ALU.add)
                nc.vector.tensor_reduce(out=s2, in_=sq, axis=AX.X, op=ALU.add)
                mu_g = work.tile([C, 1], f32, tag="st", bufs=8, name="mu_g")
                ex2_g = work.tile([C, 1], f32, tag="st", bufs=8, name="ex2_g")
                nc.vector.tensor_scalar_mul(mu_g, s1, inv_na)
                nc.vector.tensor_scalar_mul(ex2_g, s2, inv_na)
                musq_g = work.tile([C, 1], f32, tag="st", bufs=8, name="musq_g")
                nc.vector.tensor_mul(musq_g, mu_g, mu_g)
                veps_g = work.tile([C, 1], f32, tag="st", bufs=8, name="veps_g")
                nc.vector.scalar_tensor_tensor(
                    veps_g, ex2_g, EPS, musq_g, op0=ALU.add, op1=ALU.subtract
                )
                rstd_g = newton_rsqrt(veps_g, "g")
                nc.vector.tensor_scalar(
                    xn_s.rearrange("p j l -> p (j l)"),
                    xg_sb,
                    mu_g,
                    rstd_g,
                    op0=ALU.subtract,
                    op1=ALU.mult,
                )

            def xarr_pair(t):
                if "1" not in phases:
                    return
                for j in (2 * t, 2 * t + 1):
                    ps_t = pp.tile([C, B_TOTAL], f16, tag="pss", bufs=2, name="ps_tj")
                    nc.tensor.transpose(ps_t, xn_s[:, j, :], iden_sb)
                    nc.scalar.activation(
                        out=xarr[:, :, j],
                        in_=ps_t,
                        func=AF.Identity,
                        scale=gvec_sb[:, j : j + 1],
                        bias=bb_sb[:, j : j + 1],
                    )

            # calibration variant ("3" without "2"): zero aarr so the
            # assembly chunks are runnable without the attention stages
            if "2" not in phases and phases != "1":
                nc.vector.memset(aarr[:], 0.0)

            # ===== attention for the 12 local batches =====================
            st = {}

            def st1(b):
                d = st[b] = {}
                xn16 = work.tile([S, C], f16, tag="xn16", bufs=4, name="xn16")
                nc.scalar.activation(
                    out=xn16,
                    in_=xall_sb[:, b, :],
                    func=AF.Identity,
                    scale=scale_tb[:, b : b + 1],
                    bias=shift_tb[:, b : b + 1],
                )
                ps_xt = pp.tile([C, S], f16, tag="pss", bufs=2, name="ps_xt")
                nc.tensor.transpose(ps_xt, xn16, iden_sb)
                xnT = work.tile([C + 1, S], f16, tag="xnT", bufs=4, name="xnT")
                nc.scalar.activation(out=xnT[0:C, :], in_=ps_xt, func=AF.Copy)
                nc.vector.memset(xnT[C : C + 1, :], 1.0)
                d["xnT"] = xnT

            def st2(b):
                d = st[b]
                xnT = d["xnT"]
                # q and k land in one 2-bank psum pair tile per head-group
                # (q in bank 0, k in bank 1) so each eviction is a single
                # ACT op covering both; v pairs its two head-groups the
                # same way.  Halves the ACT op count of the old layout.
                qkT = work.tile([DK, 2, NH, S], f16, tag="qkT", bufs=4, name="qkT")
                v = work.tile([S, NH, DK], f16, tag="v", bufs=4, name="v")
                for hh in range(2):
                    psqk = pp.tile([DK, 1024], f32, tag="ps", bufs=2, name="ps_qk")
                    for hl in range(4):
                        h = 4 * hh + hl
                        nc.tensor.matmul(
                            psqk[:, hl * S : (hl + 1) * S],
                            lhsT=wq_sb[:, h, :],
                            rhs=xnT,
                            start=True,
                            stop=True,
                        )
                        nc.tensor.matmul(
                            psqk[:, 512 + hl * S : 512 + (hl + 1) * S],
                            lhsT=wk_sb[:, h, :],
                            rhs=xnT,
                            start=True,
                            stop=True,
                        )
                    sl = slice(4 * hh, 4 * (hh + 1))
                    nc.scalar.activation(
                        out=qkT[:, :, sl, :],
                        in_=psqk.rearrange("p (q x) -> p q x", q=2)[
                            :, :, 0:384
                        ].rearrange("p q (h s) -> p q h s", h=4),
                        func=AF.Copy,
                    )
                psv = pp.tile([S, 1024], f32, tag="ps", bufs=2, name="ps_v")
                for hh in range(2):
                    for hl in range(4):
                        h = 4 * hh + hl
                        nc.tensor.matmul(
                            psv[:, 512 * hh + hl * DK : 512 * hh + (hl + 1) * DK],
                            lhsT=xnT[0:C, :],
                            rhs=wv_sb[:, h, :],
                            start=True,
                            stop=True,
                        )
                nc.scalar.activation(
                    out=v.rearrange("p (q h) s -> p q h s", q=2),
                    in_=psv.rearrange("p (q x) -> p q x", q=2)[
                        :, :, 0:384
                    ].rearrange("p q (h s) -> p q h s", h=4),
                    func=AF.Copy,
                )
                d["qkT"], d["v"] = qkT, v

            def st3(b):
                # scoresT with the causal mask ADDED in-psum: the mask matmul
                # opens each bank's accumulation group (start=True over the
                # whole 384-col region), the per-head score matmuls then
                # accumulate into their 96-col slices.  exp of both
                # head-groups is a single ACT op over the 2-bank pair.
                d = st[b]
                qkT = d["qkT"]
                expT = work.tile([S, NH, S], f16, tag="expT", bufs=4, name="expT")
                pst = pp.tile([S, 1024], f32, tag="ps", bufs=2, name="ps_sc")
                for hh in range(2):
                    off = 512 * hh
                    nc.tensor.matmul(
                        pst[:, off : off + 384],
                        lhsT=maskb_sb,
                        rhs=iden4_sb[:].rearrange("c h q -> c (h q)"),
                        start=True,
                        stop=False,
                    )
                    for hl in range(4):
                        h = 4 * hh + hl
                        nc.tensor.matmul(
                            pst[:, off + hl * S : off + (hl + 1) * S],
                            lhsT=qkT[:, 1, h, :],
                            rhs=qkT[:, 0, h, :],
                            start=False,
                            stop=(hl == 3),
                        )
                nc.scalar.activation(
                    out=expT.rearrange("p (q h) s -> p q h s", q=2),
                    in_=pst.rearrange("p (q x) -> p q x", q=2)[
                        :, :, 0:384
                    ].rearrange("p q (h s) -> p q h s", h=4),
                    func=AF.Exp,
                )
                d["expT"] = expT

            def st4(b):
                d = st[b]
                expT = d["expT"]
                recip = work.tile([S, NH, S], f16, tag="recip", bufs=4, name="recip")
                psd = pp.tile([S, 1024], f32, tag="ps", bufs=2, name="ps_den")
                for hh in range(2):
                    nc.tensor.matmul(
                        psd[:, 512 * hh : 512 * hh + 384],
                        lhsT=ones_sb,
                        rhs=expT[:, 4 * hh : 4 * (hh + 1), :].rearrange(
                            "p h s -> p (h s)"
                        ),
                        start=True,
                        stop=True,
                    )
                with nc.allow_low_precision(
                    reason="attn weights in f16; 2e-2 gate"
                ):
                    nc.vector.reciprocal(
                        out=recip.rearrange("p (q h) s -> p q h s", q=2),
                        in_=psd.rearrange("p (q x) -> p q x", q=2)[
                            :, :, 0:384
                        ].rearrange("p q (h s) -> p q h s", h=4),
                    )
                d["recip"] = recip

            def st5(b):
                d = st.pop(b)
                ocatT = work.tile([DK, NH, S], f16, tag="ocatT", name="ocatT")
                pso = pp.tile([DK, 1024], f32, tag="ps", bufs=2, name="ps_o")
                for hh in range(2):
                    for hl in range(4):
                        h = 4 * hh + hl
                        nc.tensor.matmul(
                            pso[:, 512 * hh + hl * S : 512 * hh + (hl + 1) * S],
                            lhsT=d["v"][:, h, :],
                            rhs=d["expT"][:, h, :],
                            start=True,
                            stop=True,
                        )
                # softmax normalize folded into the PSUM eviction (recip is
                # replicated across partitions by the ones-matmul). DVE since
                # GPSIMD cannot read PSUM.
                nc.vector.tensor_tensor(
                    ocatT.rearrange("p (q h) s -> p q h s", q=2),
                    pso.rearrange("p (q x) -> p q x", q=2)[
                        :, :, 0:384
                    ].rearrange("p q (h s) -> p q h s", h=4),
                    d["recip"].rearrange("p (q h) s -> p q h s", q=2),
                    ALU.mult,
                )
                # A_jT[l, k] = sum_h Wo_h^T @ oT_h  (+ bo_eff via bb in xarr)
                psw = pp.tile([C, S], f32, tag="psw", bufs=2, name="ps_w")
                for h in range(NH):
                    nc.tensor.matmul(
                        psw,
                        lhsT=wo_sb[:, h, :],
                        rhs=ocatT[:, h, :],
                        start=(h == 0),
                        stop=(h == NH - 1),
                    )
                nc.scalar.activation(out=aarr[:, :, b], in_=psw, func=AF.Copy)

            # ===== assembly: out[l, i, k, j2] = A + xn, fp16, j-pair t =====
            # Each i-half chunk is split between DVE (rows < ASPL, f16 2x
            # packed mode) and GpSimd (rows >= ASPL) so both engines chew on
            # it concurrently; one DMA per chunk once both parts land.
            dummies = []
            dma_variant = any(x in phases for x in "567")
            if dma_variant:
                nd = 3 if "7" in phases else 1
                for ii in range(nd):
                    dm = cpool.tile(
                        [C, B_TOTAL // 2, S, 2], f16, name=f"dummy_res{ii}"
                    )
                    nc.vector.memset(dm[:], 0.25)
                    dummies.append(dm)

            # probes "a"/"b"/"c": per-j 3D ops (S3S3D3-compatible)
            if "a" in phases or "b" in phases or "c" in phases:
                pa2 = cpool.tile([C, JPC, S], f16, name="probe_a2")  # [l, j, k]
                px2 = cpool.tile([C, B_TOTAL, JPC], f16, name="probe_x2")
                nc.vector.memset(pa2[:], 0.5)
                nc.vector.memset(px2[:], 0.25)
                for j in range(JPC):
                    res3 = work.tile([C, B_TOTAL, S], f16, tag="res", bufs=3, name="res3")
                    if "a" in phases:
                        # TT: in0 contiguous, in1 mid-dim stride-0 bcast,
                        # all innermost stride 1 -> 2x_1P candidate
                        nc.vector.tensor_tensor(
                            res3,
                            res3,
                            pa2[:, j, :].unsqueeze(1).to_broadcast((C, B_TOTAL, S)),
                            ALU.add,
                        )
                    elif "c" in phases:
                        # per-j single-pass: in0 i-bcast (innermost 1),
                        # in1 k-bcast (innermost 0), both 3D
                        nc.vector.tensor_tensor(
                            res3,
                            pa2[:, j, :].unsqueeze(1).to_broadcast((C, B_TOTAL, S)),
                            px2[:, :, j].unsqueeze(2).to_broadcast((C, B_TOTAL, S)),
                            ALU.add,
                        )
                    else:
                        # copy with innermost stride-0 src (k-expansion)
                        nc.vector.tensor_copy(
                            out=res3,
                            in_=px2[:, :, j].unsqueeze(2).to_broadcast(
                                (C, B_TOTAL, S)
                            ),
                        )

            # TT-mode probes: "8" = contiguous f16 TT same volume; "9" =
            # broadcast TT with 4-wide innermost (j-quads)
            if "8" in phases or "9" in phases:
                pa = cpool.tile([C, S, JPC], f16, name="probe_a")
                px = cpool.tile([C, B_TOTAL, JPC], f16, name="probe_x")
                psrc = cpool.tile([C, B_TOTAL // 2 * S * 2], f16, name="probe_src")
                nc.vector.memset(pa[:], 0.5)
                nc.vector.memset(px[:], 0.25)
                nc.vector.memset(psrc[:], 0.125)
                for t in range(NJP):
                    for ih in range(2):
                        res = work.tile(
                            [C, B_TOTAL // 2, S, 2], f16, tag="res", bufs=3, name="res"
                        )
                        if "8" in phases:
                            nc.vector.tensor_tensor(
                                res.rearrange("p a b c -> p (a b c)"),
                                psrc[:],
                                psrc[:],
                                ALU.add,
                            )
                        else:
                            q = t // 2
                            ab = pa[:, :, 4 * q : 4 * q + 4].unsqueeze(1)
                            xb_ = px[:, ih * 24 : ih * 24 + 24, 4 * q : 4 * q + 4].unsqueeze(2)
                            nc.vector.tensor_tensor(
                                res.rearrange("p a b c -> p (a b c)").rearrange(
                                    "p (x y z) -> p x y z", y=S, z=4
                                ),
                                ab.to_broadcast((C, 24, S, 4)),
                                xb_.to_broadcast((C, 24, S, 4)),
                                ALU.add,
                            )

            def asm_chunk(t, ih):
                if "3" not in phases and "4" not in phases and not dma_variant:
                    return
                if dma_variant:
                    src = dummies[(2 * t + ih) % len(dummies)]
                    ov = out_d[t][:, ih * 48 : (ih + 1) * 48, :, :]
                    if "6" in phases:
                        nc.sync.dma_start(
                            out=ov.bitcast(f32), in_=src[:].bitcast(f32)
                        )
                    else:
                        nc.sync.dma_start(out=ov, in_=src)
                    return
                res = work.tile(
                    [C, B_TOTAL // 2, S, 2], f16, tag="res", bufs=3, name="res"
                )
                ab = aarr[:, :, 2 * t : 2 * t + 2].unsqueeze(1)
                xb_ = xarr[:, ih * 48 : (ih + 1) * 48, 2 * t : 2 * t + 2].unsqueeze(2)
                nc.vector.tensor_tensor(
                    res[:, 0:aspl, :, :],
                    ab.to_broadcast((C, aspl, S, 2)),
                    xb_[:, 0:aspl].to_broadcast((C, aspl, S, 2)),
                    ALU.add,
                )
                if aspl < 48:
                    nc.gpsimd.tensor_tensor(
                        res[:, aspl:48, :, :],
                        ab.to_broadcast((C, 48 - aspl, S, 2)),
                        xb_[:, aspl:48].to_broadcast((C, 48 - aspl, S, 2)),
                        ALU.add,
                    )
                if "4" not in phases:
                    nc.sync.dma_start(
                        out=out_d[t][:, ih * 48 : (ih + 1) * 48, :, :], in_=res
                    )

            # Software pipeline: assembly of pair t-1 is emitted at the top
            # of pair t so DVE/GpSimd chew on it while pair t's chain runs
            # on ACT/PE, and the out-DMAs pace one pair behind.
            if "2" not in phases and ("3" in phases or "4" in phases or dma_variant):
                for t in range(NJP):
                    xarr_pair(t)
                    asm_chunk(t, 0)
                    asm_chunk(t, 1)
            if "2" in phases:
                # groups of 4 batches, stage-interleaved; assembly of the
                # previous group's two pairs at the top of each group
                for g in range(NJP // 2 - 1):
                    xarr_pair(2 * g)
                    xarr_pair(2 * g + 1)
                    if g > 0:
                        for tp in (2 * g - 2, 2 * g - 1):
                            asm_chunk(tp, 0)
                            asm_chunk(tp, 1)
                    bs = [4 * g + i for i in range(4)]
                    for fn in (st1, st2, st3, st4, st5):
                        for b in bs:
                            fn(b)
                # last group pair-wise so asm of its first pair overlaps the
                # second pair's tail; only asm(NJP-1) is fully exposed
                xarr_pair(NJP - 2)
                xarr_pair(NJP - 1)
                for tp in (NJP - 4, NJP - 3):
                    asm_chunk(tp, 0)
                    asm_chunk(tp, 1)
                b0 = 2 * (NJP - 2)
                for fn in (st1, st2, st3, st4, st5):
                    fn(b0)
                    fn(b0 + 1)
                b1 = 2 * (NJP - 1)
                st1(b1)
                st1(b1 + 1)
                st2(b1)
                st2(b1 + 1)
                asm_chunk(NJP - 2, 0)
                asm_chunk(NJP - 2, 1)
                st3(b1)
                st3(b1 + 1)
                st4(b1)
                st4(b1 + 1)
                st5(b1)
                st5(b1 + 1)
                asm_chunk(NJP - 1, 0)
                asm_chunk(NJP - 1, 1)

            loop_cm.__exit__(None, None, None)

    nc.compile()
    return nc


def _get_program():
    global _PROG
    if _PROG is None:
        _PROG = _build_program()
    return _PROG


def _host_inputs(x, Wq, bq, Wk, bk, Wv, bv, Wo, bo, gamma, beta):
    f32 = np.float32
    f16 = np.float16
    x = np.asarray(x, f32)
    Wq = np.asarray(Wq, f32)
    bq = np.asarray(bq, f32)
    Wk = np.asarray(Wk, f32)
    bk = np.asarray(bk, f32)
    Wv = np.asarray(Wv, f32)
    bv = np.asarray(bv, f32)
    Wo = np.asarray(Wo, f32)
    bo = np.asarray(bo, f32)
    gamma = np.asarray(gamma, f32)
    beta = np.asarray(beta, f32)

    sc = f32(1.0 / np.sqrt(DK))
    wq97 = np.concatenate(
        [(Wq * sc).reshape(C, NH, DK), (bq * sc).reshape(1, NH, DK)], axis=0
    )
    wk97 = np.concatenate([Wk.reshape(C, NH, DK), bk.reshape(1, NH, DK)], axis=0)
    bo_eff = (bv.astype(np.float64) @ Wo.astype(np.float64) + bo).astype(f32)

    cp16 = np.zeros((C + 1, 3840), f16)
    cp16[:, 0:768] = wq97.reshape(C + 1, 768).astype(f16)
    cp16[:, 768:1536] = wk97.reshape(C + 1, 768).astype(f16)
    cp16[0:C, 1536:2304] = Wv.reshape(C, 768).astype(f16)
    cp16[0:DK, 2304:3072] = (
        Wo.reshape(NH, DK, C).transpose(1, 0, 2).reshape(DK, 768).astype(f16)
    )
    cp16[0:C, 3072:3168] = np.kron(
        np.eye(G, dtype=f16), np.ones((C // G, C // G), f16)
    )
    cp16[0:S, 3168:3264] = np.ones((S, S), f16)
    # maskbT[q, t] = -30 where t > q (causal), added to scoresT in-psum
    cp16[0:S, 3264:3360] = np.triu(np.full((S, S), -30.0, f16), 1)
    cp16[0:C, 3360:3744] = np.broadcast_to(
        np.eye(C, dtype=f16)[:, None, :], (C, 4, S)
    ).reshape(C, 384)
    cp16[0:C, 3744:3840] = np.eye(C, dtype=f16)

    com = {"cpack16": cp16}
    x_r = np.ascontiguousarray(x.reshape(B_TOTAL, C, C))
    in_maps = []
    for c in range(NCORES):
        J = slice(c * JPC, (c + 1) * JPC)
        m = dict(com)
        # [s, b, c] f16 so the one upfront DMA lands as SBUF [s | (b, c)]
        m["xall"] = np.ascontiguousarray(
            x_r[J].transpose(1, 0, 2).astype(f16)
        )
        m["xg"] = (
            np.ascontiguousarray(x_r[:, J, :])
            .reshape(B_TOTAL, JPC * C)
            .astype(f16)
        )
        cp32 = np.zeros((C, 26), f32)
        cp32[:, 0] = gamma
        cp32[:, 1] = beta
        cp32[:, 2:14] = np.broadcast_to(gamma[J][None, :], (C, JPC))
        cp32[:, 14:26] = beta[J][None, :] + bo_eff[:, None]
        m["cpack32"] = cp32
        in_maps.append(m)
    return in_maps


def _assemble(parts):
    """parts[c]: (NJP, C, B, S, 2) f16 [jp, l, i, k, j2] -> (B, C, S, C) f32."""
    cols = []
    for a in parts:
        a = np.asarray(a).astype(np.float32).reshape(NJP, C, B_TOTAL, S, 2)
        # (jp, l, i, k, j2) -> (i, jp, j2, k, l)
        a = a.transpose(2, 0, 4, 3, 1).reshape(B_TOTAL, JPC, S, C)
        cols.append(a)
    return np.concatenate(cols, axis=1)


def _run(inputs, trace=False):
    from concourse.bass_utils import run_bass_kernel_spmd

    nc = _get_program()
    in_maps = _host_inputs(**inputs)
    res = run_bass_kernel_spmd(
        nc, in_maps, core_ids=list(range(NCORES)), trace=trace
    )
    out = _assemble([r["out"] for r in res.results])
    return out, res


def kernel(**inputs) -> np.ndarray:
    out, _ = _run(inputs, trace=False)
    return out



# revision 9
# speedup vs baseline: 1.3392x; 1.3392x over previous
"""Trainium2 Bass kernel for nn_Attention_43542378447097 (v3).

GroupNorm -> multi-head causal self-attention -> out-proj, then the
reference's broadcast add:

    out(B,S,C) + residual(B,C,1,C)  ->  (B,C,S,C)

    result[i, j, k, l] = A[j, k, l] + xn[i, j, l]

where A[j] = attention output (no bo) of batch j and xn[i] = groupnorm
output of batch i (+ beta + bo_eff folded in).  Output (96,96,96,96),
fp16 on device = 21.2MB/core across 8 j-sharded cores -> the out-DMA
(~71us on the scalar HWDGE ring) is the intended critical path.

v3 vs v2 (measured on HW via probe phases):
  * Assembly TT reshaped to j-quad innermost [l, i24, k96, j4]: DVE
    hits ~2.2 elem/cyc/partition (54.7us total) vs 1.2 for j-pairs
    (96us).  aarr is [l, k, j] and xarr [l, i, j] so both operands
    stream innermost step-1 quads; broadcasts sit on mid dims only.
  * Out-DMAs issued on nc.scalar (qActDynamicHW ring): 71.2us vs
    84.6us on nc.sync for the same 12 x 1.77MB transfers.
  * Attention restructured around xallT [c, b, s] (host-transposed):
    groupnorm scale folds into the contraction as xsc98 = xallT *
    scale2 with two extra matmul rows (shift-row, ones-row) carrying
    the shift*Wsum and bias terms, so q/k/v need no per-batch
    transposes or ACT scale passes.  q/k are batched over all 12
    local batches per head (N=384 matmuls).
  * Softmax: denominators via ones-matmul (replicated across
    partitions), reciprocal via the custom DVE op (~1 cyc/elem, f16
    out), and the normalize multiply runs on the otherwise-idle
    GPSIMD engine from SBUF (ACT evicts raw oT).
  * ACT only ever loads the exp_and_others table (Copy/Identity/Exp/
    Square); rsqrt stays the all-DVE Newton chain.
"""

import sys

sys.path.insert(0, "/opt/trn_rl_repo")

import numpy as np

B_TOTAL = 96
C = 96
S = 96
NH = 8
DK = 96
G = 8
NCORES = 8
JPC = B_TOTAL // NCORES  # 12 local j's (attention batches) per core
JW = 4  # assembly quad width (j's per asm group)
NJG = JPC // JW  # 3 quad groups
IW = 96 // JW  # 24 i's per asm op
EPS = 1e-5

_PROG = None


def _build_program(loop_n=1, phases="123", aspl=None, dma_ring="scalar"):
    import contextlib

    import concourse.bass as bass
    import concourse.tile as tile
    from concourse import bacc, mybir
    from concourse.dve_ops import RECIP_APPROX_FAST_CONSTS, RECIPROCAL_APPROX_FAST

    f32 = mybir.dt.float32
    f16 = mybir.dt.float16
    i32 = mybir.dt.int32
    AF = mybir.ActivationFunctionType
    ALU = mybir.AluOpType
    AX = mybir.AxisListType

    nc = bacc.Bacc(
        "TRN2",
        target_bir_lowering=False,
        debug=False,
        enable_asserts=False,
        num_devices=NCORES,
    )

    xallT_d = nc.declare_dram_parameter("xallT", [C, JPC, S], f16, isOutput=False)
    xg_d = nc.declare_dram_parameter("xg", [B_TOTAL, JPC * C], f16, isOutput=False)
    cp16_d = nc.declare_dram_parameter("cpack16", [98, 3936], f16, isOutput=False)
    cp32_d = nc.declare_dram_parameter("cpack32", [98, 216], f32, isOutput=False)
    out_d = nc.declare_dram_parameter(
        "out", [NJG, JW, C, IW, S, JW], f16, isOutput=True
    )

    with tile.TileContext(nc) as tc:
        with (
            tc.tile_pool(name="const", bufs=1) as cpool,
            tc.tile_pool(name="work", bufs=2) as work,
            tc.tile_pool(name="psum", bufs=8, space="PSUM") as pp,
        ):
            # ---- constants (packed tiles; views below) ----
            cp16 = cpool.tile([98, 3936], f16, name="cp16")
            cp32 = cpool.tile([98, 216], f32, name="cp32")
            aarr = cpool.tile([C, S, JPC], f16, name="aarr")  # [l, k, j]
            xarr = cpool.tile([C, B_TOTAL, JPC], f16, name="xarr")  # [l, i, j]
            xsc98 = cpool.tile([98, JPC, S], f16, name="xsc98")
            qkT = cpool.tile([DK, 2, NH, JPC, S], f16, name="qkT")

            wq_sb = cp16[:, 0:768].rearrange("p (h d) -> p h d", h=NH)
            wk_sb = cp16[:, 768:1536].rearrange("p (h d) -> p h d", h=NH)
            wv_sb = cp16[0:97, 1536:2304]
            wo_sb = cp16[0:DK, 2304:3072].rearrange("p (h l) -> p h l", h=NH)
            maskb_sb = cp16[0:S, 3072:3168]
            ones_sb = cp16[0:S, 3168:3264]
            iden4_sb = cp16[0:C, 3264:3648]
            iden_sb = cp16[0:C, 3648:3744]
            ones98_sb = cp16[0:C, 3744:3842]
            gvec_sb = cp32[0:C, 0:12]
            bb_sb = cp32[0:C, 12:24]
            gamma_rep = cp32[:, 24:120]
            beta_rep = cp32[:, 120:216]

            nc.sync.dma_start(out=cp16, in_=cp16_d[:])
            nc.sync.dma_start(out=cp32, in_=cp32_d[:])
            # rows 96-97 = 1.0 once; the loop overwrites row 96 with the
            # shift row each iteration (engines need 32-aligned start
            # partitions, so the two rows are set together)
            nc.vector.memset(xsc98[96:98, :, :], 1.0)

            inv_na = 1.0 / (C * C // G)  # 1/1152 per (batch, group)
            rc = RECIP_APPROX_FAST_CONSTS

            loop_cm = (
                tc.For_i(0, loop_n, 1) if loop_n > 1 else contextlib.nullcontext()
            )
            loop_cm.__enter__()

            def newton_rsqrt(veps, tag, iters=2):
                """rstd = rsqrt(veps), all-DVE (quake seed + Newton steps)
                so ACT only ever needs the Exp table set."""
                shp = list(veps.shape)
                iv = veps.bitcast(i32)
                ineg = work.tile(shp, i32, tag="sti", bufs=8, name="ineg" + tag)
                nc.vector.tensor_scalar_mul(ineg, iv, -1)
                nc.vector.tensor_scalar(ineg, ineg, 1, None, op0=ALU.arith_shift_right)
                nc.vector.tensor_scalar(ineg, ineg, 0x5F3759DF, None, op0=ALU.add)
                y = ineg.bitcast(f32)
                t1 = work.tile(shp, f32, tag="st", bufs=8, name="t1" + tag)
                for _ in range(iters):
                    nc.vector.tensor_mul(t1, y, y)
                    nc.vector.tensor_mul(t1, t1, veps)
                    nc.vector.tensor_scalar(t1, t1, -0.5, 1.5, op0=ALU.mult, op1=ALU.add)
                    nc.vector.tensor_mul(y, y, t1)
                return y

            # ===== phase 2 prologue: groupnorm stats over xallT, xsc98 ====
            xallT_sb = cpool.tile([C, JPC, S], f16, name="xallT_sb")
            if "2" in phases:
                nc.sync.dma_start(out=xallT_sb, in_=xallT_d[:])
                x2t = work.tile([C, JPC, S], f16, tag="x2t", name="x2t")
                nc.scalar.activation(out=x2t, in_=xallT_sb, func=AF.Square)
                s1v = work.tile([98, JPC, G], f32, tag="stv", bufs=8, name="s1v")
                s2v = work.tile([98, JPC, G], f32, tag="stv", bufs=8, name="s2v")
                for g3 in range(3):
                    bs = slice(4 * g3, 4 * (g3 + 1))
                    ps1 = pp.tile([98, 512], f32, tag="pss", bufs=2, name="ps_s1")
                    nc.tensor.matmul(
                        ps1[:, 0:384],
                        lhsT=ones98_sb,
                        rhs=xallT_sb[:, bs, :].rearrange("p b s -> p (b s)"),
                        start=True,
                        stop=True,
                    )
                    nc.vector.tensor_reduce(
                        out=s1v[:, bs, :].unsqueeze(3),
                        in_=ps1[:, 0:384].rearrange("p (b g s) -> p b g s", b=4, g=G),
                        axis=AX.X,
                        op=ALU.add,
                    )
                    ps2 = pp.tile([98, 512], f32, tag="pss", bufs=2, name="ps_s2")
                    nc.tensor.matmul(
                        ps2[:, 0:384],
                        lhsT=ones98_sb,
                        rhs=x2t[:, bs, :].rearrange("p b s -> p (b s)"),
                        start=True,
                        stop=True,
                    )
                    nc.vector.tensor_reduce(
                        out=s2v[:, bs, :].unsqueeze(3),
                        in_=ps2[:, 0:384].rearrange("p (b g s) -> p b g s", b=4, g=G),
                        axis=AX.X,
                        op=ALU.add,
                    )
                mu = work.tile([98, JPC, G], f32, tag="stv", bufs=8, name="mu")
                ex2 = work.tile([98, JPC, G], f32, tag="stv", bufs=8, name="ex2")
                nc.vector.tensor_scalar_mul(mu, s1v, inv_na)
                nc.vector.tensor_scalar_mul(ex2, s2v, inv_na)
                musq = work.tile([98, JPC, G], f32, tag="stv", bufs=8, name="musq")
                nc.vector.tensor_mul(musq, mu, mu)
                veps = work.tile([98, JPC, G], f32, tag="stv", bufs=8, name="veps")
                nc.vector.scalar_tensor_tensor(
                    veps, ex2, EPS, musq, op0=ALU.add, op1=ALU.subtract
                )
                y = newton_rsqrt(veps, "a", iters=2)
                # scale2[p, b, s] = rstd[b, g(s)] * gamma[s]; f16 so the
                # xsc multiply below runs in DVE 2x packed mode
                scale2 = work.tile([98, JPC, S], f16, tag="sc2", name="scale2")
                nc.vector.tensor_tensor(
                    scale2.rearrange("p b (g s) -> p b g s", g=G),
                    y.unsqueeze(3).to_broadcast((98, JPC, G, 12)),
                    gamma_rep.rearrange("p (g s) -> p g s", g=G)
                    .unsqueeze(1)
                    .to_broadcast((98, JPC, G, 12)),
                    ALU.mult,
                )
                # shift2 = beta - mu*scale2 (row 96 feeds the matmul
                # shift-row; rows 0-95 are computed but unused)
                msc = work.tile([98, JPC, S], f16, tag="msc", name="msc")
                nc.vector.tensor_tensor(
                    msc.rearrange("p b (g s) -> p b g s", g=G),
                    mu.unsqueeze(3).to_broadcast((98, JPC, G, 12)),
                    scale2.rearrange("p b (g s) -> p b g s", g=G),
                    ALU.mult,
                )
                shift2 = work.tile([98, JPC, S], f16, tag="sh2", name="shift2")
                nc.vector.tensor_tensor(
                    shift2,
                    beta_rep.unsqueeze(1).to_broadcast((98, JPC, S)),
                    msc,
                    ALU.subtract,
                )
                nc.vector.tensor_mul(xsc98[0:96, :, :], xallT_sb, scale2[0:96, :, :])
                nc.vector.tensor_copy(
                    out=xsc98[96:97, :, :], in_=shift2[96:97, :, :]
                )

            # ===== phase 1: xarr (xn of ALL 96 batches at this core's
            # 12 j-rows, transposed to [l, i, j]) =========================
            xn_s = cpool.tile([B_TOTAL, JPC, C], f16, name="xn_s")
            if "1" in phases:
                xg_sb = work.tile([B_TOTAL, JPC * C], f16, tag="xg", name="xg_sb")
                nc.sync.dma_start(out=xg_sb, in_=xg_d[:])
                sq = work.tile([B_TOTAL, JPC * C], f16, tag="sq", name="sq")
                nc.vector.tensor_mul(sq, xg_sb, xg_sb)
                s1 = work.tile([C, 1], f32, tag="st", bufs=8, name="s1g")
                s2 = work.tile([C, 1], f32, tag="st", bufs=8, name="s2g")
                nc.vector.tensor_reduce(out=s1, in_=xg_sb, axis=AX.X, op=ALU.add)
                nc.vector.tensor_reduce(out=s2, in_=sq, axis=AX.X, op=ALU.add)
                mu_g = work.tile([C, 1], f32, tag="st", bufs=8, name="mu_g")
                ex2_g = work.tile([C, 1], f32, tag="st", bufs=8, name="ex2_g")
                nc.vector.tensor_scalar_mul(mu_g, s1, inv_na)
                nc.vector.tensor_scalar_mul(ex2_g, s2, inv_na)
                musq_g = work.tile([C, 1], f32, tag="st", bufs=8, name="musq_g")
                nc.vector.tensor_mul(musq_g, mu_g, mu_g)
                veps_g = work.tile([C, 1], f32, tag="st", bufs=8, name="veps_g")
                nc.vector.scalar_tensor_tensor(
                    veps_g, ex2_g, EPS, musq_g, op0=ALU.add, op1=ALU.subtract
                )
                rstd_g = newton_rsqrt(veps_g, "g")
                nc.vector.tensor_scalar(
                    xn_s.rearrange("p j l -> p (j l)"),
                    xg_sb,
                    mu_g,
                    rstd_g,
                    op0=ALU.subtract,
                    op1=ALU.mult,
                )

            def xarr_quad(g):
                if "1" not in phases:
                    return
                for j in range(JW * g, JW * (g + 1)):
                    ps_t = pp.tile(
                        [C, B_TOTAL], f16, tag="pstr", bufs=1, name="ps_tj"
                    )
                    nc.tensor.transpose(ps_t, xn_s[:, j, :], iden_sb)
                    nc.scalar.activation(
                        out=xarr[:, :, j],
                        in_=ps_t,
                        func=AF.Identity,
                        scale=gvec_sb[:, j : j + 1],
                        bias=bb_sb[:, j : j + 1],
                    )

            # ===== qk phase: q/k for all 12 batches, per head ============
            if "2" in phases:
                xsc_bs = xsc98.rearrange("p b s -> p (b s)")
                for h in range(NH):
                    for qi, wsb in ((0, wq_sb), (1, wk_sb)):
                        tA = pp.tile([DK, 1024], f32, tag="ps", bufs=2, name="ps_qk")
                        nc.tensor.matmul(
                            tA[:, 0:384],
                            lhsT=wsb[:, h, :],
                            rhs=xsc_bs[:, 0:384],
                            start=True,
                            stop=True,
                        )
                        nc.tensor.matmul(
                            tA[:, 512:896],
                            lhsT=wsb[:, h, :],
                            rhs=xsc_bs[:, 384:768],
                            start=True,
                            stop=True,
                        )
                        nc.scalar.activation(
                            out=qkT[:, qi, h, 0:8, :].rearrange("p b s -> p (b s)")
                            .rearrange("p (u x) -> p u x", u=2),
                            in_=tA.rearrange("p (u x) -> p u x", u=2)[:, :, 0:384],
                            func=AF.Copy,
                        )
                        tB = pp.tile([DK, 1024], f32, tag="ps", bufs=2, name="ps_qk2")
                        nc.tensor.matmul(
                            tB[:, 0:384],
                            lhsT=wsb[:, h, :],
                            rhs=xsc_bs[:, 768:1152],
                            start=True,
                            stop=True,
                        )
                        nc.scalar.activation(
                            out=qkT[:, qi, h, 8:12, :].rearrange("p b s -> p (b s)"),
                            in_=tB[:, 0:384],
                            func=AF.Copy,
                        )

            # ===== attention stages for the 12 local batches ==============
            st = {}

            def st_v(b):
                d = st[b] = {}
                psv = pp.tile([S, 1024], f32, tag="ps", bufs=2, name="ps_v")
                nc.tensor.matmul(
                    psv[:, 0:384],
                    lhsT=xsc98[0:97, b, :],
                    rhs=wv_sb[:, 0:384],
                    start=True,
                    stop=True,
                )
                nc.tensor.matmul(
                    psv[:, 512:896],
                    lhsT=xsc98[0:97, b, :],
                    rhs=wv_sb[:, 384:768],
                    start=True,
                    stop=True,
                )
                v = work.tile([S, NH, DK], f16, tag="v", bufs=4, name="v")
                nc.scalar.activation(
                    out=v.rearrange("p (u h) d -> p u (h d)", u=2),
                    in_=psv.rearrange("p (u x) -> p u x", u=2)[:, :, 0:384],
                    func=AF.Copy,
                )
                d["v"] = v

            def st_s(b):
                d = st[b]
                pst = pp.tile([S, 1024], f32, tag="ps", bufs=2, name="ps_sc")
                for hh in range(2):
                    off = 512 * hh
                    nc.tensor.matmul(
                        pst[:, off : off + 384],
                        lhsT=maskb_sb,
                        rhs=iden4_sb,
                        start=True,
                        stop=False,
                    )
                    for hl in range(4):
                        h = 4 * hh + hl
                        nc.tensor.matmul(
                            pst[:, off + hl * S : off + (hl + 1) * S],
                            lhsT=qkT[:, 1, h, b, :],
                            rhs=qkT[:, 0, h, b, :],
                            start=False,
                            stop=(hl == 3),
                        )
                expT = work.tile([S, NH, S], f16, tag="expT", bufs=4, name="expT")
                nc.scalar.activation(
                    out=expT.rearrange("p (u h) s -> p u (h s)", u=2),
                    in_=pst.rearrange("p (u x) -> p u x", u=2)[:, :, 0:384],
                    func=AF.Exp,
                )
                d["expT"] = expT

            def st_d(b):
                d = st[b]
                expT = d["expT"]
                psd = pp.tile([S, 1024], f32, tag="ps", bufs=2, name="ps_den")
                for hh in range(2):
                    nc.tensor.matmul(
                        psd[:, 512 * hh : 512 * hh + 384],
                        lhsT=ones_sb,
                        rhs=expT[:, 4 * hh : 4 * (hh + 1), :].rearrange(
                            "p h s -> p (h s)"
                        ),
                        start=True,
                        stop=True,
                    )
                recip = work.tile([S, NH, S], f16, tag="recip", bufs=4, name="recip")
                with nc.allow_low_precision(reason="softmax recip f16; 2e-2 gate"):
                    for hh in range(2):
                        nc.vector._custom_dve(
                            RECIPROCAL_APPROX_FAST,
                            out=recip[:, 4 * hh : 4 * (hh + 1), :].rearrange(
                                "p h s -> p (h s)"
                            ),
                            in0=psd[:, 512 * hh : 512 * hh + 384],
                            s0=rc["s0"],
                            s1=rc["s1"],
                            imm2=rc["imm2"],
                        )
                d["recip"] = recip

            def st_o(b):
                d = st[b]
                pso = pp.tile([DK, 1024], f32, tag="ps", bufs=2, name="ps_o")
                for hh in range(2):
                    for hl in range(4):
                        h = 4 * hh + hl
                        nc.tensor.matmul(
                            pso[:, 512 * hh + hl * S : 512 * hh + (hl + 1) * S],
                            lhsT=d["v"][:, h, :],
                            rhs=d["expT"][:, h, :],
                            start=True,
                            stop=True,
                        )
                oraw = work.tile([DK, NH, S], f16, tag="oraw", bufs=4, name="oraw")
                nc.scalar.activation(
                    out=oraw.rearrange("p (u h) s -> p u (h s)", u=2),
                    in_=pso.rearrange("p (u x) -> p u x", u=2)[:, :, 0:384],
                    func=AF.Copy,
                )
                ocatT = work.tile([DK, NH, S], f16, tag="ocatT", bufs=4, name="ocatT")
                with nc.allow_low_precision(reason="attn weights f16; 2e-2 gate"):
                    nc.gpsimd.tensor_tensor(ocatT, oraw, d["recip"], ALU.mult)
                d["ocatT"] = ocatT

            def st_w(b):
                d = st.pop(b)
                psw = pp.tile([C, S], f32, tag="psw", bufs=1, name="ps_w")
                for h in range(NH):
                    nc.tensor.matmul(
                        psw,
                        lhsT=wo_sb[:, h, :],
                        rhs=d["ocatT"][:, h, :],
                        start=(h == 0),
                        stop=(h == NH - 1),
                    )
                nc.scalar.activation(out=aarr[:, :, b], in_=psw, func=AF.Copy)

            # ===== assembly quad g: res[l, i24, k, j4] = A + X ===========
            dma_eng = nc.scalar if dma_ring == "scalar" else nc.sync

            def asm_quad(g):
                if "3" not in phases and "4" not in phases:
                    return
                js = slice(JW * g, JW * (g + 1))
                for ic in range(JW):
                    res = work.tile(
                        [C, IW, S, JW], f16, tag="res", bufs=3, name="res"
                    )
                    nc.vector.tensor_tensor(
                        res,
                        aarr[:, :, js].unsqueeze(1).to_broadcast((C, IW, S, JW)),
                        xarr[:, ic * IW : (ic + 1) * IW, js].unsqueeze(2).to_broadcast(
                            (C, IW, S, JW)
                        ),
                        ALU.add,
                    )
                    if "4" not in phases:
                        dma_eng.dma_start(out=out_d[g, ic], in_=res)

            # ===== schedule: quads of 4 batches, stage-interleaved; the
            # assembly of quad g-1 is emitted inside quad g's stages so
            # DVE chews on it while PE/ACT run quad g ====================
            if "2" in phases:
                for g in range(NJG):
                    xarr_quad(g)
                    bs = [4 * g + i for i in range(4)]
                    for fn in (st_v, st_s, st_d):
                        for b in bs:
                            fn(b)
                    if g > 0:
                        asm_quad(g - 1)
                    for fn in (st_o, st_w):
                        for b in bs:
                            fn(b)
                asm_quad(NJG - 1)
            elif "1" in phases or "3" in phases or "4" in phases:
                # assembly/DMA timing variants without attention
                nc.vector.memset(aarr[:], 0.0)
                for g in range(NJG):
                    xarr_quad(g)
                    asm_quad(g)

            # DMA probe "z": out-DMA only, from one dummy buffer
            if "z" in phases:
                dm = cpool.tile([C, IW, S, JW], f16, name="dummy_res")
                nc.vector.memset(dm[:], 0.25)
                for g in range(NJG):
                    for ic in range(JW):
                        dma_eng.dma_start(out=out_d[g, ic], in_=dm)

            loop_cm.__exit__(None, None, None)

    nc.compile()
    return nc


def _get_program():
    global _PROG
    if _PROG is None:
        _PROG = _build_program()
    return _PROG


def _host_inputs(x, Wq, bq, Wk, bk, Wv, bv, Wo, bo, gamma, beta):
    f32 = np.float32
    f16 = np.float16
    x = np.asarray(x, f32)
    Wq = np.asarray(Wq, f32)
    bq = np.asarray(bq, f32)
    Wk = np.asarray(Wk, f32)
    bk = np.asarray(bk, f32)
    Wv = np.asarray(Wv, f32)
    bv = np.asarray(bv, f32)
    Wo = np.asarray(Wo, f32)
    bo = np.asarray(bo, f32)
    gamma = np.asarray(gamma, f32)
    beta = np.asarray(beta, f32)

    sc = f32(1.0 / np.sqrt(DK))
    bo_eff = (bv.astype(np.float64) @ Wo.astype(np.float64) + bo).astype(f32)

    cp16 = np.zeros((98, 3936), f16)
    cp16[0:96, 0:768] = (Wq * sc).astype(f16)
    cp16[96, 0:768] = (Wq.sum(axis=0) * sc).astype(f16)
    cp16[97, 0:768] = (bq * sc).astype(f16)
    cp16[0:96, 768:1536] = Wk.astype(f16)
    cp16[96, 768:1536] = Wk.sum(axis=0).astype(f16)
    cp16[97, 768:1536] = bk.astype(f16)
    cp16[0:96, 1536:2304] = Wv.astype(f16)
    cp16[96, 1536:2304] = Wv.sum(axis=0).astype(f16)
    cp16[0:96, 2304:3072] = (
        Wo.reshape(NH, DK, C).transpose(1, 0, 2).reshape(DK, 768).astype(f16)
    )
    # maskbT[q, t] = -30 where t > q (causal), added to scoresT in-psum
    cp16[0:S, 3072:3168] = np.triu(np.full((S, S), -30.0, f16), 1)
    cp16[0:S, 3168:3264] = np.ones((S, S), f16)
    cp16[0:C, 3264:3648] = np.broadcast_to(
        np.eye(C, dtype=f16)[:, None, :], (C, 4, S)
    ).reshape(C, 384)
    cp16[0:C, 3648:3744] = np.eye(C, dtype=f16)
    cp16[0:C, 3744:3842] = np.ones((C, 98), f16)

    com = {"cpack16": cp16}
    x_r = np.ascontiguousarray(x.reshape(B_TOTAL, C, C))
    in_maps = []
    for c in range(NCORES):
        J = slice(c * JPC, (c + 1) * JPC)
        m = dict(com)
        # xallT[c, b, s] = x[local b, s(chan), c(w)] transposed
        m["xallT"] = np.ascontiguousarray(
            x_r[J].transpose(2, 0, 1).astype(f16)
        )
        m["xg"] = (
            np.ascontiguousarray(x_r[:, J, :])
            .reshape(B_TOTAL, JPC * C)
            .astype(f16)
        )
        cp32 = np.zeros((98, 216), f32)
        cp32[0:C, 0:12] = np.broadcast_to(gamma[J][None, :], (C, JPC))
        cp32[0:C, 12:24] = beta[J][None, :] + bo_eff[:, None]
        cp32[:, 24:120] = np.broadcast_to(gamma[None, :], (98, C))
        cp32[:, 120:216] = np.broadcast_to(beta[None, :], (98, C))
        m["cpack32"] = cp32
        in_maps.append(m)
    return in_maps


def _assemble(parts):
    """parts[c]: (NJG, JW, C, IW, S, JW) f16 [jq, ic, l, i24, k, j4]
    -> (B, C, S, C) f32."""
    cols = []
    for a in parts:
        a = np.asarray(a).astype(np.float32).reshape(NJG, JW, C, IW, S, JW)
        # (jq, ic, l, i24, k, j4) -> (ic, i24, jq, j4, k, l)
        a = a.transpose(1, 3, 0, 5, 4, 2).reshape(B_TOTAL, JPC, S, C)
        cols.append(a)
    return np.concatenate(cols, axis=1)


def _run(inputs, trace=False):
    from concourse.bass_utils import run_bass_kernel_spmd

    nc = _get_program()
    in_maps = _host_inputs(**inputs)
    res = run_bass_kernel_spmd(
        nc, in_maps, core_ids=list(range(NCORES)), trace=trace
    )
    out = _assemble([r["out"] for r in res.results])
    return out, res


def kernel(**inputs) -> np.ndarray:
    out, _ = _run(inputs, trace=False)
    return out
